# revision 31
# baseline (speedup 1.0000x reference)
"""GCAttention (channel-add) Trainium2 kernel — bf16 data path.

Data-parallel over batch: 32 batches -> 8 NeuronCores x 4 batches each.

The rel-err budget (2e-2 of absmax) admits bf16 for x and y: the host
converts x -> bf16 (error ~2^-9 rel, measured end-to-end 5.7e-3) and the
device reads/writes bf16, halving HBM traffic vs f32 (25.7MB/core) and
making every PE matmul 1 cycle/row (f32 is 4) plus DVE 2x/4x modes.

Per batch (C=512, S=56*56=3136), x_b (3.2MB bf16) is SBUF-resident,
loaded in 3 group-aligned column pieces so compute starts as data lands:

  mask phase (pipelined one batch ahead of the heavy ctx phase):
  1. eb[s] = exp(wm.x[:,s] + bm): the stationary operand is wm
     pre-REPLICATED on the host into [128,128] blocks, so the mask
     matmul lands in PSUM already broadcast across all 128 partitions
     and ONE ACT exp per 512-slice writes eb [128,S] bf16 directly
     (no partition-broadcast step at all). |mask|<~7, no max-sub.
     The fused ACT accum gives per-partition Z -> 1/Z via two tiny
     DVE ops, off the critical chain.
  ctx phase:
  2. ctx[c] = sum_s x[c,s]*eb[s]: fused DVE scalar_tensor_tensor is
     pinned at 1x mode, so chunks 0-1 (+2 partially) run as stts on
     DVE split by column group, while chunk 3 (and chunk 2's widest
     group) go hybrid: bf16 TT-multiply on DVE (2x mode) + ACT
     accum-reduce. Partial sums land in ctxp[128, group, chunk].
  mlp phase — engineered as a short cross-engine chain because the
  Tile list-scheduler stretches every serial hop by whatever ready
  work that engine has (a DVE hop costs ~1.7us of interleaved stt):
  3. u    = w1 @ ctx   (tiny accumulating PE matmuls, one per ctxp
            partial — linearity absorbs the group splits)
  4. hid  = u*(1/Z) + b1  (ONE ACT op: per-partition scale/bias,
            [64,1] column layout)
  5. h1   = LayerNorm(hid)*g + b  (ONE fused GPSIMD layernorm,
            n_tokens=2; partitions 0-63 are the real token)
  6. hT   = relu(h1) into a [65,1] tile whose row 64 is constant 1
  7. addg = [gamma*w2T ; gamma*b2] @ hT  (4 PE matmuls; the 65th
            contraction row folds in the bias; gamma folded on host)
  8. x   += addg in place (chunk 0 ACT, chunks 1-3 DVE 4x-mode);
     each chunk is DMA-stored the moment its add lands.

All params are packed on the host into two blobs (replicated-wm bf16,
f32 everything-else with w1/w2 pre-transposed and gamma/b2 pre-folded)
so setup is 2 tiny DMAs ahead of the x loads. Single SP HWDGE ring,
program order: params, L0..L3 pieces, then per-chunk stores in compute
order — loads are never stuck behind a waiting store.

Measured on the axon trn2 cores: ~100us/iter (vs 211.6us f32
baseline), rel err 5.7e-3. A DMA-only ablation runs at ~39us
(~650GB/s/core!), so the kernel is bound by the DVE/ACT elementwise
streams (ctx + adds), not by HBM bandwidth.
"""

import sys

import numpy as np

try:
    import concourse.bacc as bacc
except ImportError:  # grading env may not have concourse on sys.path
    sys.path.insert(0, "/opt/trn_rl_repo")
    import concourse.bacc as bacc

import concourse.tile as tile
from concourse import bass_utils, mybir

F32 = mybir.dt.float32
BF16 = mybir.dt.bfloat16
OP = mybir.AluOpType
AF = mybir.ActivationFunctionType

B, C, H, W = 32, 512, 56, 56
S = H * W  # 3136
P = 64
EPS = 1e-3
N_CORES = 8
B_LOC = B // N_CORES  # 4
NCH = C // 128  # 4 channel chunks
QW = S // 4  # 784, load quarter width
# s-slices sized to one PSUM bank (512 fp32): 6x512 + 64
SLICES = [(j * 512, min(512, S - j * 512)) for j in range((S + 511) // 512)]
NSL = len(SLICES)  # 7
# eb broadcast / ctx column groups: slice 0 | slices 1-2 | slices 3-6
# (first group small so batch-0 ctx starts as soon as possible)
GROUPS = [(0, 512), (512, 1024), (1536, S - 1536)]

# f32 param blob layout: [128, PF]
W1OFF = 0          # w1T [128, 4*64]  (w1T[p, k*64+f] = w1[f, k*128+p])
W2OFF = 256        # [gamma*w2T ; gamma*b2] on partitions 0..64: [65, 512]
B1TOFF = 768       # b1 column [64, 1]
GLNOFF = 769       # ln_g column [64, 1] (zeros on 64..127)
BLNOFF = 770       # ln_b column [64, 1]
BMOFF = 771        # bm [1, 1]
PF = 772

_CACHE: dict = {}
_LN_STUB = False
_DMA_ONLY = False


def _build(loops: int = 0, timing: bool = False):
    nc = bacc.Bacc(
        "TRN2", target_bir_lowering=False, debug=False, num_devices=N_CORES
    )
    if timing:
        # timing-only build: no huge host transfers, x is device garbage
        nc.dram_tensor("din", [8], F32, kind="ExternalInput").ap()
        x = nc.dram_tensor("x", [B_LOC, C, S], BF16, kind="Internal").ap()
    else:
        x = nc.dram_tensor("x", [B_LOC, C, S], BF16, kind="ExternalInput").ap()
    wm = nc.dram_tensor("wm", [128, NCH * 128], BF16, kind="ExternalInput").ap()
    pf32 = nc.dram_tensor("pf32", [128, PF], F32, kind="ExternalInput").ap()
    if timing:
        y = nc.dram_tensor("yint", [B_LOC, C, S], BF16, kind="Internal").ap()
        yout = nc.dram_tensor("y", [1, 8], F32, kind="ExternalOutput").ap()
    else:
        y = nc.dram_tensor("y", [B_LOC, C, S], BF16, kind="ExternalOutput").ap()
        yout = None

    with tile.TileContext(nc) as tc:
        from contextlib import ExitStack

        with ExitStack() as ctx:
            consts = ctx.enter_context(tc.tile_pool(name="consts", bufs=1))
            xpool = ctx.enter_context(tc.tile_pool(name="xpool", bufs=4))
            epool = ctx.enter_context(tc.tile_pool(name="epool", bufs=2))
            ebpool = ctx.enter_context(tc.tile_pool(name="ebpool", bufs=2))
            prodpool = ctx.enter_context(tc.tile_pool(name="prodpool", bufs=2))
            small = ctx.enter_context(tc.tile_pool(name="small", bufs=2))
            zpool = ctx.enter_context(tc.tile_pool(name="zpool", bufs=2))
            mask_ps = ctx.enter_context(
                tc.tile_pool(name="mask_ps", bufs=3, space="PSUM")
            )
            mlp_ps = ctx.enter_context(
                tc.tile_pool(name="mlp_ps", bufs=2, space="PSUM")
            )
            addg_pool = ctx.enter_context(
                tc.tile_pool(name="addg_ps", bufs=2, space="PSUM")
            )

            # ---- params (2 small DMAs, ahead of the x loads) --------------
            wm_sb = consts.tile([128, NCH * 128], BF16)
            nc.sync.dma_start(wm_sb[:, :], wm)
            pf = consts.tile([128, PF], F32)
            nc.sync.dma_start(pf[:, :], pf32)
            if timing:
                tout = consts.tile([1, 8], F32)
                nc.vector.memset(tout[:, :], 1.0)
                nc.sync.dma_start(yout[:, :], tout[:, :])

            ones_row = consts.tile([1, P], F32)
            nc.vector.memset(ones_row[:, :], 1.0)
            # hT65: row 64 stays constant 1.0 (folds gamma*b2 into the
            # addg matmul); rows 0..63 rewritten by relu each batch
            hT65 = consts.tile([P + 1, 1], F32)
            nc.vector.memset(hT65[:, :], 1.0)
            # layernorm input [128,1]: only partitions 0..63 are the token;
            # 64..127 pre-set to 1.0 so token-1 stats stay finite
            hid_in = consts.tile([128, 1], F32)
            nc.vector.memset(hid_in[:, :], 1.0)

            x_tiles = []
            for b in range(B_LOC):
                x_tiles.append(
                    xpool.tile([128, NCH, S], BF16, tag="x", name=f"xt{b}")
                )

            def load_batch(b):
                xb = x[b].rearrange("(k p) s -> p k s", p=128)
                for g0, gw in GROUPS:
                    nc.sync.dma_start(
                        x_tiles[b][:, :, g0 : g0 + gw],
                        xb[:, :, g0 : g0 + gw],
                    )

            # per-batch state produced by mask_phase, consumed by ctx/mlp
            state = {}

            def mask_phase(b):
                # mask matmuls use host-replicated wm as the stationary
                # operand, so the mask (and exp of it) comes out already
                # broadcast across all 128 partitions: eb = exp() directly,
                # no partition-broadcast, and Z is per-partition for free
                x_t = x_tiles[b]
                zp = zpool.tile([128, 8], F32, tag="zp")
                eb_sb = ebpool.tile([128, S], BF16, tag="eb")
                for j, (s0, sw) in enumerate(SLICES):
                    mps = mask_ps.tile([128, 512], F32, tag="mask")
                    for k in range(NCH):
                        nc.tensor.matmul(
                            mps[:, :sw],
                            lhsT=wm_sb[:, k * 128 : (k + 1) * 128],
                            rhs=x_t[:, k, s0 : s0 + sw],
                            start=(k == 0),
                            stop=(k == NCH - 1),
                        )
                    nc.scalar.activation(
                        eb_sb[:, s0 : s0 + sw],
                        mps[:, :sw],
                        AF.Exp,
                        bias=pf[:, BMOFF : BMOFF + 1],
                        scale=1.0,
                        accum_out=zp[:, j : j + 1],
                    )
                # Z -> 1/Z per partition (tiny DVE ops, off the hot chain)
                z_col = small.tile([128, 1], F32, tag="z")
                nc.vector.reduce_sum(
                    z_col[:, :], zp[:, 0:NSL], axis=mybir.AxisListType.X
                )
                zrb_sb = small.tile([128, 1], F32, tag="zrb_sb")
                nc.vector.reciprocal(zrb_sb[:, :], z_col[:, :])
                state[b] = (eb_sb, zrb_sb)

            def ctx_dve_part(b):
                x_t = x_tiles[b]
                eb_sb, zrb_sb = state[b]
                prod = prodpool.tile([128, S], BF16, tag="prod")
                ctxp = small.tile([128, len(GROUPS), NCH], F32, tag="ctxp")
                for g, (g0, gw) in enumerate(GROUPS):
                    for k in range(2):
                        nc.vector.scalar_tensor_tensor(
                            out=prod[:, g0 : g0 + gw],
                            in0=x_t[:, k, g0 : g0 + gw],
                            scalar=1.0,
                            in1=eb_sb[:, g0 : g0 + gw],
                            op0=OP.bypass,
                            op1=OP.mult,
                            accum_out=ctxp[:, g, k : k + 1],
                        )
                # chunks 2,3 hybrid: bf16 TT-mult on DVE (2x mode) writes
                # the product, ACT reduces it via accum (cheaper than the
                # 1x-mode fused stt when split across the two engines)
                for k, (h0, hw) in ((2, (0, S)), (3, (0, S))):
                    prodh = prodpool.tile([128, S], BF16, tag=f"prod{k}")
                    nc.vector.tensor_mul(
                        prodh[:, h0 : h0 + hw],
                        x_t[:, k, h0 : h0 + hw],
                        eb_sb[:, h0 : h0 + hw],
                    )
                    nc.scalar.activation(
                        prodh[:, h0 : h0 + hw],
                        prodh[:, h0 : h0 + hw],
                        AF.Identity,
                        scale=1.0,
                        accum_out=ctxp[:, 0, k : k + 1],
                    )
                state[b] = (eb_sb, zrb_sb, ctxp)

            def mlp_phase(b):
                x_t = x_tiles[b]
                _, zrb_sb, ctxp = state.pop(b)

                # u = w1 @ ctx: 8 tiny accumulating matmuls (one per ctxp
                # partial; linearity absorbs the half-split)
                u_ps = mlp_ps.tile([P, 1], F32, tag="mlp")
                pieces = [
                    (g, k)
                    for g in range(len(GROUPS))
                    for k in range(NCH)
                    if k < 2 or g == 0
                ]
                for n, (g, k) in enumerate(pieces):
                    nc.tensor.matmul(
                        u_ps[:, :],
                        lhsT=pf[:, W1OFF + k * P : W1OFF + (k + 1) * P],
                        rhs=ctxp[:, g, k : k + 1],
                        start=(n == 0),
                        stop=(n == len(pieces) - 1),
                    )
                # hid = u/Z + b1 (one ACT op, per-partition scale and bias)
                nc.scalar.activation(
                    hid_in[0:P, 0:1],
                    u_ps[:, :],
                    AF.Identity,
                    bias=pf[0:P, B1TOFF : B1TOFF + 1],
                    scale=zrb_sb[0:P, 0:1],
                )
                # fused LayerNorm (token = partitions 0..63)
                h1 = small.tile([128, 1], F32, tag="h1")
                if _LN_STUB:
                    nc.scalar.copy(h1[:, :], hid_in[:, :])
                else:
                    nc.gpsimd.layernorm(
                        h1[:, :],
                        hid_in[:, :],
                        gamma_ap=pf[:, GLNOFF : GLNOFF + 1],
                        beta_ap=pf[:, BLNOFF : BLNOFF + 1],
                        eps=EPS,
                        subtract_mean=True,
                        n_tokens=2,
                    )
                # relu into the [65,1] rhs (row 64 stays 1.0)
                nc.scalar.activation(
                    hT65[0:P, 0:1], h1[0:P, 0:1], AF.Relu, scale=1.0
                )

                # addg[c] = [gamma*w2T ; gamma*b2] @ hT65 (bias via row 64)
                addg_ps = addg_pool.tile([128, NCH], F32, tag="addg")
                for k in range(NCH):
                    nc.tensor.matmul(
                        addg_ps[:, k : k + 1],
                        lhsT=pf[0 : P + 1, W2OFF + k * 128 : W2OFF + (k + 1) * 128],
                        rhs=hT65[:, :],
                        start=True,
                        stop=True,
                    )
                addg = small.tile([128, NCH], F32, tag="addg")
                nc.scalar.copy(addg[:, :], addg_ps[:, :])

                # x += addg in place (chunk 0 ACT, 1-3 DVE 4x-mode), and
                # store each chunk as soon as its add lands
                yb = y[b].rearrange("(k p) s -> p k s", p=128)
                for k in range(NCH):
                    if k == 0:
                        nc.scalar.activation(
                            x_t[:, k, :],
                            x_t[:, k, :],
                            AF.Identity,
                            bias=addg[:, k : k + 1],
                            scale=1.0,
                        )
                    else:
                        nc.vector.tensor_scalar_add(
                            x_t[:, k, :], x_t[:, k, :], addg[:, k : k + 1]
                        )
                    nc.sync.dma_start(yb[:, k, :], x_t[:, k, :])

            def store_batch(b):
                pass

            def pipeline():
                for b in range(B_LOC):
                    load_batch(b)
                if _DMA_ONLY:
                    for b in range(B_LOC):
                        store_batch(b)
                    return
                mask_phase(0)
                for b in range(B_LOC):
                    ctx_dve_part(b)
                    mlp_phase(b)
                    store_batch(b)
                    if b + 1 < B_LOC:
                        mask_phase(b + 1)

            if loops:
                with tc.For_i(0, loops, 1):
                    pipeline()
            else:
                pipeline()

    nc.compile()
    return nc


def _get_nc():
    if "nc" not in _CACHE:
        _CACHE["nc"] = _build()
    return _CACHE["nc"]


def _pack_wm(wm_flat):
    import ml_dtypes

    # wm_bc[p, k*128+i] = wm[k*128+p] (stationary operand pre-replicated so
    # the mask matmul output is broadcast across partitions)
    return np.ascontiguousarray(
        np.repeat(
            wm_flat.reshape(NCH, 128, 1).transpose(1, 0, 2), 128, axis=2
        ).reshape(128, NCH * 128)
    ).astype(ml_dtypes.bfloat16)


def _pack_params(w1, b1, ln_g, ln_b, w2, b2, bm, gamma):
    blob = np.zeros((128, PF), np.float32)
    # w1T[p, k*64+f] = w1[f, k*128+p]
    blob[:, W1OFF : W1OFF + NCH * P] = (
        w1.reshape(P, NCH, 128).transpose(2, 1, 0).reshape(128, NCH * P)
    )
    blob[0:P, W2OFF : W2OFF + C] = gamma[0] * w2.T
    blob[P, W2OFF : W2OFF + C] = gamma[0] * b2
    blob[0:P, B1TOFF] = b1
    blob[0:P, GLNOFF] = ln_g
    blob[0:P, BLNOFF] = ln_b
    blob[:, BMOFF] = bm[0]
    return blob


def kernel(**inputs) -> np.ndarray:
    import ml_dtypes

    x = np.asarray(inputs["x"], np.float32).reshape(B, C, S)
    x_bf = np.ascontiguousarray(x.astype(ml_dtypes.bfloat16))
    wm = _pack_wm(np.asarray(inputs["wm"], np.float32).reshape(C))
    blob = _pack_params(
        np.asarray(inputs["w1"], np.float32),
        np.asarray(inputs["b1"], np.float32),
        np.asarray(inputs["ln_g"], np.float32),
        np.asarray(inputs["ln_b"], np.float32),
        np.asarray(inputs["w2"], np.float32),
        np.asarray(inputs["b2"], np.float32),
        np.asarray(inputs["bm"], np.float32).reshape(1),
        np.asarray(inputs["gamma"], np.float32).reshape(1),
    )

    nc = _get_nc()
    shared = {"wm": wm, "pf32": blob}
    in_maps = [
        {"x": x_bf[c * B_LOC : (c + 1) * B_LOC], **shared}
        for c in range(N_CORES)
    ]
    res = bass_utils.run_bass_kernel_spmd(
        nc, in_maps, core_ids=list(range(N_CORES)), **_CACHE.get("run_kwargs", {})
    )
    _CACHE["last_results"] = res
    out = np.concatenate(
        [res.results[c]["y"] for c in range(N_CORES)], axis=0
    )
    return out.astype(np.float32).reshape(B, C, H, W)
